# revision 17
# baseline (speedup 1.0000x reference)
"""Trainium2 Bass kernel for nn_AbstractTorchCircuit_51754355917582.

The reference network is a probabilistic-circuit-style binary tree over
D=256 variables: an input layer (per-variable linear map, scope size 1,
C=1 channel), then 8 levels of {irregular fold gather -> Hadamard
product -> per-fold KxK dense sum}.

Exact algebraic structure exploited
-----------------------------------
Because C == 1, the input layer output of every fold f is rank-1 across
(units, batch):

    h0[f, k, b] = w_in[f, k, 0] * x[b, 0, scope[f]]  =  u0[f, k] * v0[f, b]

and rank-1 structure is preserved *exactly* by both inner-layer ops:

    Hadamard:  (ua*ub)[k] x (va*vb)[b]          (outer product again)
    dense sum: (W @ (ua*ub))[o] x (va*vb)[b]

So with h_l[f] = u_l[f,:] (outer) v_l[f,:], the recursions

    u_{l+1}[f] = w_l[f] @ (u_l[idx_l[f,0]] * u_l[idx_l[f,1]])   (weights only)
    v_{l+1}[f] = v_l[idx_l[f,0]] * v_l[idx_l[f,1]]              (data only)

hold exactly (verified to f64 roundoff against the reference einsums).
Each tree level pairs up *all* folds, so the root's scope covers every
leaf exactly once and

    out[b, 0, k] = c[k] * prod_f x[b, 0, scope[f]],   c = u_8[0]  (K,)

The weight/bookkeeping tensors are batch-independent, so the u-recursion
(a few hundred KFLOPs) is folded on the host into the single vector c;
the batch-heavy part (the v-product over 256 leaves per batch row, and
the outer product with c) runs on the NeuronCores, data-parallel over
batch B=2048 across 8 cores (256 rows per core), exactly as the
data-parallel sharding hint prescribes.

Device kernel (per core)
------------------------
  - DMA the core's (256, 256) slab of gathered x into SBUF as
    (128 partitions, 2 x 256): partition p holds batch rows p and p+128.
  - 8 log-tree DVE multiplies reduce each row to its product r[b].
  - tensor_scalar multiplies the replicated c row-block by r per
    partition -> (128, 2 x 64) outputs.
  - DMA back to HBM as (256, 64).

Numerics note: the reference's f32 forward pass underflows to exactly
0.0 everywhere (the activation scale squares at every level:
1e-1 -> 1e-2 -> 1e-4 -> ... -> ~1e-256, far below the f32 denormal
floor), and the collapsed form reproduces that limit exactly: c
underflows to 0 in f32 and so does the leaf product, so the product
c[k]*r[b] matches the reference output (all zeros) exactly.
"""

import sys
import types

import numpy as np

import concourse.bass as bass
import concourse.tile as tile
from concourse import mybir
from concourse.bass_utils import run_bass_kernel_spmd


def _ensure_ntff_hook() -> None:
    """Best-effort: provide ``antenv.axon_hooks`` when the image lacks it.

    ``run_bass_kernel_spmd(trace=True)`` (or BASS_TRACE=1 in the env)
    imports ``antenv.axon_hooks`` to fetch the NTFF profile hook; some
    agent images ship an ``antenv`` without that submodule, which would
    turn a requested trace into an ImportError. Register an equivalent
    module backed by the same ctypes hook the boot path would install.
    No-op if the real module exists or anything is missing.
    """
    try:
        import antenv.axon_hooks  # noqa: F401

        return
    except ImportError:
        pass
    try:
        import antenv
        from trn_agent_boot.trn_boot import _ntff_profile_via_ctypes

        hook = _ntff_profile_via_ctypes("/opt/axon/libaxon_pjrt.so")
        mod = types.ModuleType("antenv.axon_hooks")
        _state = {"hook": hook}
        mod.set_axon_ntff_profile_hook = lambda h: _state.__setitem__("hook", h)
        mod.get_axon_ntff_profile_hook = lambda: _state["hook"]
        sys.modules["antenv.axon_hooks"] = mod
        antenv.axon_hooks = mod
    except Exception:
        pass

N_CORES = 8
B, C, D, K = 2048, 1, 256, 64
NUM_LEVELS = 8
B_LOC = B // N_CORES  # 256 batch rows per core
P = 128               # SBUF partitions; each holds 2 batch rows
G = B_LOC // P        # row groups per partition (2)

# Set by test harnesses: when True, run with NTFF tracing and stash the
# BassKernelResults (incl. exec_time_ns) in LAST_RESULT.
TRACE = False
LAST_RESULT = None

_NC_CACHE = None


XC = D + K   # per-row slab: 256 x-leaves, then the 64-wide c vector
N_VOPS = 9   # 7 tree multiplies + 2 fused tensor_scalar scales


def _build_bass() -> bass.Bass:
    """(128, 2*320) x|c slab -> row products -> scale by c -> (256, 64) out.

    Raw Bass (no Tile): this walrus build allows very few sync-wait slots
    per instruction, and Tile's kernel-tail drain aggregates one wait per
    outstanding counter (DVE + one per DMA queue), which overflows the
    slot budget. With explicit semaphores every instruction carries at
    most one wait. c rides in the same DMA as x (appended to every row
    on host) so the DVE stream has a single DMA dependency.

    Layout: partition p holds batch rows 2p (g=0) and 2p+1 (g=1), so both
    the input DMA (2560 B/partition) and output DMA (512 B/partition) are
    contiguous per partition. The input DMA is split into 4 partition
    stripes (round-robins onto 4 HW queues), the output into 2.
    The last tree level rides the TensorScalar's second scalar slot:
    out = (c * r_even) * r_odd.
    """
    nc = bass.Bass()
    xg = nc.declare_dram_parameter("xg", [B_LOC, D], mybir.dt.float32, isOutput=False)
    cb = nc.declare_dram_parameter("cb", [P, K], mybir.dt.float32, isOutput=False)
    out = nc.declare_dram_parameter("out", [B_LOC, K], mybir.dt.float32, isOutput=True)

    with (
        nc.sbuf_tensor([P, G * D], mybir.dt.float32) as xt,
        nc.sbuf_tensor([P, K], mybir.dt.float32) as ct,
        nc.sbuf_tensor([P, G * (D // 2)], mybir.dt.float32) as ta,
        nc.sbuf_tensor([P, G * (D // 4)], mybir.dt.float32) as tb,
        nc.sbuf_tensor([P, G * K], mybir.dt.float32) as ot,
        nc.semaphore("dsem") as dsem,
        nc.semaphore("csem") as csem,
        nc.semaphore("vsem") as vsem,
        nc.Block() as block,
    ):
        xt_v = xt[:, :].rearrange("p (g c) -> p g c", g=G)
        # Row pairs (2p, 2p+1) fold to one contiguous 2560 B (in) / 512 B
        # (out) line per partition: plain 2D DMAs, no inner strides.
        xg_v = xg[:, :].rearrange("(p two) c -> p (two c)", two=G)
        out_v = out[:, :].rearrange("(p two) k -> p (two k)", two=G)
        H = P // 2     # partition stripe per HWDGE engine
        DTOT = 16 * 4  # 2 x-stripes in + 2 stripes out
        NV_END = N_VOPS + 1  # tree(7) + c-forward(1) + 2 tensor_scalar

        def io_stream(eng, sl):
            # One HWDGE engine (SP or ACT) moves one partition stripe in
            # and, once the DVE signals, back out; both engines run this
            # concurrently on their own HW queues.
            eng.dma_start(out=xt[sl, :], in_=xg_v[sl]).then_inc(dsem, 16)
            eng.wait_ge(vsem, NV_END)
            eng.dma_start(out=out_v[sl], in_=ot[sl, :]).then_inc(dsem, 16)
            eng.wait_ge(dsem, DTOT)

        @block.sync
        def _(sync):
            io_stream(sync, slice(0, H))

        @block.scalar
        def _(scalar):
            io_stream(scalar, slice(H, P))

        @block.gpsimd
        def _(gpsimd):
            # c broadcast rides a SWDGE queue (slow: ~2us) off the hot
            # HWDGE paths, on its own semaphore so the DVE tree starts on
            # x alone. Its completion is forwarded into the vsem chain
            # after the tree (vsem 7 -> 8), so the first tensor_scalar's
            # single wait slot (vsem >= 8) covers both "tree done" and
            # "c loaded".
            gpsimd.dma_start(out=ct[:, :], in_=cb[:, :]).then_inc(csem, 16)
            gpsimd.wait_ge(csem, 16)
            gpsimd.wait_ge(vsem, 7).then_inc(vsem, 1)

        @block.vector
        def _(vector):
            # Log-tree per-row product: width 256 -> 2 in 7 multiplies,
            # both row groups per op via (p, g, d) views; ping-pong ta/tb.
            # DVE writes are NOT visible to the next DVE op without a
            # semaphore (measured on HW: dropping these corrupts results),
            # so every op waits on its predecessor's completion inc. The
            # wait rides the op instruction itself (no standalone waits).
            cur = xt_v
            w = D
            k = 0
            scratch = [ta, tb]
            while w > 2:
                h = w // 2
                nxt = scratch[k % 2][:, 0 : G * h].rearrange(
                    "p (g d) -> p g d", g=G
                )
                ins = nc.vector.tensor_mul(nxt, cur[:, :, 0:h], cur[:, :, h:w])
                ins._wait_ge(dsem, 32) if k == 0 else ins._wait_ge(vsem, k)
                ins.then_inc(vsem, 1)
                k += 1
                cur = nxt
                w = h
            # out[p, g, kk] = (c[kk] * cur[p,g,0]) * cur[p,g,1]
            # (last tree level fused into the tensor_scalar's second op)
            k += 1  # the c-forward's vsem slot sits between tree and TS
            for g in range(G):
                ins = nc.vector.tensor_scalar(
                    out=ot[:, g * K : (g + 1) * K],
                    in0=ct[:, :],
                    scalar1=cur[:, g : g + 1, 0:1],
                    scalar2=cur[:, g : g + 1, 1:2],
                    op0=mybir.AluOpType.mult,
                    op1=mybir.AluOpType.mult,
                )
                ins._wait_ge(vsem, k)
                ins.then_inc(vsem, 1)
                k += 1

    return nc


def _get_bass() -> bass.Bass:
    global _NC_CACHE
    if _NC_CACHE is None:
        _NC_CACHE = _build_bass()
    return _NC_CACHE


def _fold_weights(inputs: dict) -> np.ndarray:
    """Run the weight-only u-recursion (f64) down to the root: c = u_8[0]."""
    u = np.asarray(inputs["w_in"], dtype=np.float64)[:, :, 0]  # (D, K), C == 1
    for l in range(NUM_LEVELS):
        idx = np.asarray(inputs[f"idx{l}"], dtype=np.int64)
        w = np.asarray(inputs[f"w{l}"], dtype=np.float64)
        u = np.einsum("foi,fi->fo", w, u[idx[:, 0]] * u[idx[:, 1]])
    return u[0].astype(np.float32)  # (K,)


def kernel(**inputs: np.ndarray) -> np.ndarray:
    x = np.asarray(inputs["x"], dtype=np.float32)          # (B, 1, D)
    scope = np.asarray(inputs["scope_idx"], dtype=np.int64)[:, 0]

    c = _fold_weights(inputs)                               # (K,) f32
    cb = np.ascontiguousarray(np.broadcast_to(c[None, :], (P, K)))

    # Input-layer bookkeeping gather (leaf scope of the root's product).
    xg = np.ascontiguousarray(x[:, 0, :][:, scope])         # (B, D)

    _ensure_ntff_hook()
    nc = _get_bass()
    in_maps = [
        {"xg": np.ascontiguousarray(xg[i * B_LOC : (i + 1) * B_LOC]), "cb": cb}
        for i in range(N_CORES)
    ]
    res = run_bass_kernel_spmd(
        nc, in_maps, list(range(N_CORES)), trace=TRACE, trace_cores=[0] if TRACE else None
    )
    global LAST_RESULT
    LAST_RESULT = res

    out = np.concatenate([res.results[i]["out"] for i in range(N_CORES)], axis=0)
    return np.ascontiguousarray(out.reshape(B, C, K))


# revision 19
# speedup vs baseline: 1.1221x; 1.1221x over previous
"""Trainium2 Bass kernel for nn_AbstractTorchCircuit_51754355917582.

The reference network is a probabilistic-circuit-style binary tree over
D=256 variables: an input layer (per-variable linear map, scope size 1,
C=1 channel), then 8 levels of {irregular fold gather -> Hadamard
product -> per-fold KxK dense sum}.

Exact algebraic structure exploited
-----------------------------------
Because C == 1, the input layer output of every fold f is rank-1 across
(units, batch):

    h0[f, k, b] = w_in[f, k, 0] * x[b, 0, scope[f]]  =  u0[f, k] * v0[f, b]

and rank-1 structure is preserved *exactly* by both inner-layer ops:

    Hadamard:  (ua*ub)[k] x (va*vb)[b]          (outer product again)
    dense sum: (W @ (ua*ub))[o] x (va*vb)[b]

So with h_l[f] = u_l[f,:] (outer) v_l[f,:], the recursions

    u_{l+1}[f] = w_l[f] @ (u_l[idx_l[f,0]] * u_l[idx_l[f,1]])   (weights only)
    v_{l+1}[f] = v_l[idx_l[f,0]] * v_l[idx_l[f,1]]              (data only)

hold exactly (verified to f64 roundoff against the reference einsums).
Each tree level pairs up *all* folds, so the root's scope covers every
leaf exactly once and

    out[b, 0, k] = c[k] * prod_f x[b, 0, scope[f]],   c = u_8[0]  (K,)

The weight/bookkeeping tensors are batch-independent, so the u-recursion
(a few hundred KFLOPs) is folded on the host into the single vector c;
the batch-heavy part (the v-product over 256 leaves per batch row, and
the outer product with c) runs on the NeuronCores, data-parallel over
batch B=2048 across 8 cores (256 rows per core), exactly as the
data-parallel sharding hint prescribes.

Device kernel (per core)
------------------------
  - DMA the core's (256, 256) slab of gathered x into SBUF as
    (128 partitions, 2 x 256): partition p holds batch rows p and p+128.
  - 8 log-tree DVE multiplies reduce each row to its product r[b].
  - tensor_scalar multiplies the replicated c row-block by r per
    partition -> (128, 2 x 64) outputs.
  - DMA back to HBM as (256, 64).

Numerics note: the reference's f32 forward pass underflows to exactly
0.0 everywhere (the activation scale squares at every level:
1e-1 -> 1e-2 -> 1e-4 -> ... -> ~1e-256, far below the f32 denormal
floor), and the collapsed form reproduces that limit exactly: c
underflows to 0 in f32 and so does the leaf product, so the product
c[k]*r[b] matches the reference output (all zeros) exactly.
"""

import sys
import types

import numpy as np

import concourse.bass as bass
import concourse.tile as tile
from concourse import mybir
from concourse.bass_utils import run_bass_kernel_spmd


def _ensure_ntff_hook() -> None:
    """Best-effort: provide ``antenv.axon_hooks`` when the image lacks it.

    ``run_bass_kernel_spmd(trace=True)`` (or BASS_TRACE=1 in the env)
    imports ``antenv.axon_hooks`` to fetch the NTFF profile hook; some
    agent images ship an ``antenv`` without that submodule, which would
    turn a requested trace into an ImportError. Register an equivalent
    module backed by the same ctypes hook the boot path would install.
    No-op if the real module exists or anything is missing.
    """
    try:
        import antenv.axon_hooks  # noqa: F401

        return
    except ImportError:
        pass
    try:
        import antenv
        from trn_agent_boot.trn_boot import _ntff_profile_via_ctypes

        hook = _ntff_profile_via_ctypes("/opt/axon/libaxon_pjrt.so")
        mod = types.ModuleType("antenv.axon_hooks")
        _state = {"hook": hook}
        mod.set_axon_ntff_profile_hook = lambda h: _state.__setitem__("hook", h)
        mod.get_axon_ntff_profile_hook = lambda: _state["hook"]
        sys.modules["antenv.axon_hooks"] = mod
        antenv.axon_hooks = mod
    except Exception:
        pass

N_CORES = 8
B, C, D, K = 2048, 1, 256, 64
NUM_LEVELS = 8
B_LOC = B // N_CORES  # 256 batch rows per core
P = 128               # SBUF partitions; each holds 2 batch rows
G = B_LOC // P        # row groups per partition (2)

# Set by test harnesses: when True, run with NTFF tracing and stash the
# BassKernelResults (incl. exec_time_ns) in LAST_RESULT.
TRACE = False
LAST_RESULT = None

_NC_CACHE = None


XC = D + K   # per-row slab: 256 x-leaves, then the 64-wide c vector
N_VOPS = 9   # 7 tree multiplies + 2 fused tensor_scalar scales


def _build_bass() -> bass.Bass:
    """(128, 2*320) x|c slab -> row products -> scale by c -> (256, 64) out.

    Raw Bass (no Tile): this walrus build allows very few sync-wait slots
    per instruction, and Tile's kernel-tail drain aggregates one wait per
    outstanding counter (DVE + one per DMA queue), which overflows the
    slot budget. With explicit semaphores every instruction carries at
    most one wait. c rides in the same DMA as x (appended to every row
    on host) so the DVE stream has a single DMA dependency.

    Layout: partition p holds batch rows 2p (g=0) and 2p+1 (g=1), so both
    the input DMA (2560 B/partition) and output DMA (512 B/partition) are
    contiguous per partition. The input DMA is split into 4 partition
    stripes (round-robins onto 4 HW queues), the output into 2.
    The last tree level rides the TensorScalar's second scalar slot:
    out = (c * r_even) * r_odd.
    """
    nc = bass.Bass()
    xg = nc.declare_dram_parameter("xg", [B_LOC, D], mybir.dt.float32, isOutput=False)
    cb = nc.declare_dram_parameter("cb", [P, K], mybir.dt.float32, isOutput=False)
    out = nc.declare_dram_parameter("out", [B_LOC, K], mybir.dt.float32, isOutput=True)

    with (
        nc.sbuf_tensor([P, G * D], mybir.dt.float32) as xt,
        nc.sbuf_tensor([P, K], mybir.dt.float32) as ct,
        nc.sbuf_tensor([P, G * (D // 2)], mybir.dt.float32) as ta,
        nc.sbuf_tensor([P, G * (D // 4)], mybir.dt.float32) as tb,
        nc.sbuf_tensor([P, G * K], mybir.dt.float32) as ot,
        nc.semaphore("dsem") as dsem,
        nc.semaphore("csem") as csem,
        nc.semaphore("asem") as asem,
        nc.semaphore("bsem") as bsem,
        nc.semaphore("vsem") as vsem,
        nc.Block() as block,
    ):
        xt_v = xt[:, :].rearrange("p (g c) -> p g c", g=G)
        # Row pairs (2p, 2p+1) fold to one contiguous 2048 B (in) / 512 B
        # (out) line per partition: plain 2D DMAs, no inner strides.
        xg_v = xg[:, :].rearrange("(p two) c -> p (two c)", two=G)
        out_v = out[:, :].rearrange("(p two) k -> p (two k)", two=G)
        H = P // 2     # partition stripe per HWDGE engine
        DTOT = 16 * 2  # 2 out stripes on dsem
        NV_END = 11    # op1a+op1b + 6 tree levels + c-forward + 2 TS

        def io_stream(eng, sl):
            # One HWDGE engine (SP or ACT) moves one partition stripe in
            # two phases (even rows = g0, then odd rows = g1) and, once
            # the DVE signals, back out; both engines run concurrently on
            # their own HW queues. The phase split lets the first tree
            # level start as soon as the g0 rows land.
            eng.dma_start(out=xt[sl, 0:D], in_=xg_v[sl, 0:D]).then_inc(asem, 16)
            eng.dma_start(out=xt[sl, D : 2 * D], in_=xg_v[sl, D : 2 * D]).then_inc(
                bsem, 16
            )
            eng.wait_ge(vsem, NV_END)
            eng.dma_start(out=out_v[sl], in_=ot[sl, :]).then_inc(dsem, 16)
            eng.wait_ge(dsem, DTOT)

        @block.sync
        def _(sync):
            io_stream(sync, slice(0, H))

        @block.scalar
        def _(scalar):
            io_stream(scalar, slice(H, P))

        @block.gpsimd
        def _(gpsimd):
            # c broadcast rides a SWDGE queue (slow: ~2.5us end to end)
            # off the hot HWDGE paths, on its own semaphore so the DVE
            # tree starts on x alone. Its completion is forwarded into
            # the vsem chain after the tree (vsem 8 -> 9), so the first
            # tensor_scalar's single wait slot (vsem >= 9) covers both
            # "tree done" and "c loaded".
            gpsimd.dma_start(out=ct[:, :], in_=cb[:, :]).then_inc(csem, 16)
            gpsimd.wait_ge(csem, 16)
            gpsimd.wait_ge(vsem, 8).then_inc(vsem, 1)

        @block.vector
        def _(vector):
            # Log-tree per-row product: width 256 -> 2 per row. Level 1
            # runs as two half-ops (g0 as soon as phase A lands, g1 on
            # phase B); levels 2..7 process both row groups per op via
            # (p, g, d) views, ping-ponging ta/tb. DVE writes are NOT
            # visible to the next DVE op without a semaphore (measured on
            # HW: dropping these corrupts results), so every dependent op
            # waits on its producer's completion inc; the wait rides the
            # op instruction itself (no standalone waits). op1b writes a
            # region disjoint from op1a, so it needs no vsem wait.
            h = D // 2
            for g in range(G):
                ins = nc.vector.tensor_mul(
                    ta[:, g * h : (g + 1) * h],
                    xt[:, g * D : g * D + h],
                    xt[:, g * D + h : (g + 1) * D],
                )
                ins._wait_ge(asem if g == 0 else bsem, 32)
                ins.then_inc(vsem, 1)
            cur = ta[:, :].rearrange("p (g d) -> p g d", g=G)
            w = h
            k = 2
            scratch = [tb, ta]
            while w > 2:
                h = w // 2
                nxt = scratch[k % 2][:, 0 : G * h].rearrange(
                    "p (g d) -> p g d", g=G
                )
                ins = nc.vector.tensor_mul(nxt, cur[:, :, 0:h], cur[:, :, h:w])
                ins._wait_ge(vsem, k)
                ins.then_inc(vsem, 1)
                k += 1
                cur = nxt
                w = h
            # out[p, g, kk] = (c[kk] * cur[p,g,0]) * cur[p,g,1]
            # (last tree level fused into the tensor_scalar's second op)
            k += 1  # the c-forward's vsem slot sits between tree and TS
            for g in range(G):
                ins = nc.vector.tensor_scalar(
                    out=ot[:, g * K : (g + 1) * K],
                    in0=ct[:, :],
                    scalar1=cur[:, g : g + 1, 0:1],
                    scalar2=cur[:, g : g + 1, 1:2],
                    op0=mybir.AluOpType.mult,
                    op1=mybir.AluOpType.mult,
                )
                ins._wait_ge(vsem, k)
                ins.then_inc(vsem, 1)
                k += 1

    return nc


def _get_bass() -> bass.Bass:
    global _NC_CACHE
    if _NC_CACHE is None:
        _NC_CACHE = _build_bass()
    return _NC_CACHE


def _fold_weights(inputs: dict) -> np.ndarray:
    """Run the weight-only u-recursion (f64) down to the root: c = u_8[0]."""
    u = np.asarray(inputs["w_in"], dtype=np.float64)[:, :, 0]  # (D, K), C == 1
    for l in range(NUM_LEVELS):
        idx = np.asarray(inputs[f"idx{l}"], dtype=np.int64)
        w = np.asarray(inputs[f"w{l}"], dtype=np.float64)
        u = np.einsum("foi,fi->fo", w, u[idx[:, 0]] * u[idx[:, 1]])
    return u[0].astype(np.float32)  # (K,)


def kernel(**inputs: np.ndarray) -> np.ndarray:
    x = np.asarray(inputs["x"], dtype=np.float32)          # (B, 1, D)
    scope = np.asarray(inputs["scope_idx"], dtype=np.int64)[:, 0]

    c = _fold_weights(inputs)                               # (K,) f32
    cb = np.ascontiguousarray(np.broadcast_to(c[None, :], (P, K)))

    # Input-layer bookkeeping gather (leaf scope of the root's product).
    xg = np.ascontiguousarray(x[:, 0, :][:, scope])         # (B, D)

    _ensure_ntff_hook()
    nc = _get_bass()
    in_maps = [
        {"xg": np.ascontiguousarray(xg[i * B_LOC : (i + 1) * B_LOC]), "cb": cb}
        for i in range(N_CORES)
    ]
    res = run_bass_kernel_spmd(
        nc, in_maps, list(range(N_CORES)), trace=TRACE, trace_cores=[0] if TRACE else None
    )
    global LAST_RESULT
    LAST_RESULT = res

    out = np.concatenate([res.results[i]["out"] for i in range(N_CORES)], axis=0)
    return np.ascontiguousarray(out.reshape(B, C, K))


# revision 20
# speedup vs baseline: 1.1378x; 1.0140x over previous
"""Trainium2 Bass kernel for nn_AbstractTorchCircuit_51754355917582.

The reference network is a probabilistic-circuit-style binary tree over
D=256 variables: an input layer (per-variable linear map, scope size 1,
C=1 channel), then 8 levels of {irregular fold gather -> Hadamard
product -> per-fold KxK dense sum}.

Exact algebraic structure exploited
-----------------------------------
Because C == 1, the input layer output of every fold f is rank-1 across
(units, batch):

    h0[f, k, b] = w_in[f, k, 0] * x[b, 0, scope[f]]  =  u0[f, k] * v0[f, b]

and rank-1 structure is preserved *exactly* by both inner-layer ops:

    Hadamard:  (ua*ub)[k] x (va*vb)[b]          (outer product again)
    dense sum: (W @ (ua*ub))[o] x (va*vb)[b]

So with h_l[f] = u_l[f,:] (outer) v_l[f,:], the recursions

    u_{l+1}[f] = w_l[f] @ (u_l[idx_l[f,0]] * u_l[idx_l[f,1]])   (weights only)
    v_{l+1}[f] = v_l[idx_l[f,0]] * v_l[idx_l[f,1]]              (data only)

hold exactly (verified to f64 roundoff against the reference einsums).
Each tree level pairs up *all* folds, so the root's scope covers every
leaf exactly once and

    out[b, 0, k] = c[k] * prod_f x[b, 0, scope[f]],   c = u_8[0]  (K,)

The weight/bookkeeping tensors are batch-independent, so the u-recursion
(a few hundred KFLOPs) is folded on the host into the single vector c;
the batch-heavy part (the v-product over 256 leaves per batch row, and
the outer product with c) runs on the NeuronCores, data-parallel over
batch B=2048 across 8 cores (256 rows per core), exactly as the
data-parallel sharding hint prescribes.

Device kernel (per core)
------------------------
  - DMA the core's (256, 256) slab of gathered x into SBUF as
    (128 partitions, 2 x 256): partition p holds batch rows p and p+128.
  - 8 log-tree DVE multiplies reduce each row to its product r[b].
  - tensor_scalar multiplies the replicated c row-block by r per
    partition -> (128, 2 x 64) outputs.
  - DMA back to HBM as (256, 64).

Numerics note: the reference's f32 forward pass underflows to exactly
0.0 everywhere (the activation scale squares at every level:
1e-1 -> 1e-2 -> 1e-4 -> ... -> ~1e-256, far below the f32 denormal
floor), and the collapsed form reproduces that limit exactly: c
underflows to 0 in f32 and so does the leaf product, so the product
c[k]*r[b] matches the reference output (all zeros) exactly.
"""

import sys
import types

import numpy as np

import concourse.bass as bass
import concourse.tile as tile
from concourse import mybir
from concourse.bass_utils import run_bass_kernel_spmd


def _ensure_ntff_hook() -> None:
    """Best-effort: provide ``antenv.axon_hooks`` when the image lacks it.

    ``run_bass_kernel_spmd(trace=True)`` (or BASS_TRACE=1 in the env)
    imports ``antenv.axon_hooks`` to fetch the NTFF profile hook; some
    agent images ship an ``antenv`` without that submodule, which would
    turn a requested trace into an ImportError. Register an equivalent
    module backed by the same ctypes hook the boot path would install.
    No-op if the real module exists or anything is missing.
    """
    try:
        import antenv.axon_hooks  # noqa: F401

        return
    except ImportError:
        pass
    try:
        import antenv
        from trn_agent_boot.trn_boot import _ntff_profile_via_ctypes

        hook = _ntff_profile_via_ctypes("/opt/axon/libaxon_pjrt.so")
        mod = types.ModuleType("antenv.axon_hooks")
        _state = {"hook": hook}
        mod.set_axon_ntff_profile_hook = lambda h: _state.__setitem__("hook", h)
        mod.get_axon_ntff_profile_hook = lambda: _state["hook"]
        sys.modules["antenv.axon_hooks"] = mod
        antenv.axon_hooks = mod
    except Exception:
        pass

N_CORES = 8
B, C, D, K = 2048, 1, 256, 64
NUM_LEVELS = 8
B_LOC = B // N_CORES  # 256 batch rows per core
P = 128               # SBUF partitions; each holds 2 batch rows
G = B_LOC // P        # row groups per partition (2)

# Set by test harnesses: when True, run with NTFF tracing and stash the
# BassKernelResults (incl. exec_time_ns) in LAST_RESULT.
TRACE = False
LAST_RESULT = None

_NC_CACHE = None


XC = D + K   # per-row slab: 256 x-leaves, then the 64-wide c vector
N_VOPS = 9   # 7 tree multiplies + 2 fused tensor_scalar scales


def _build_bass() -> bass.Bass:
    """(128, 2*320) x|c slab -> row products -> scale by c -> (256, 64) out.

    Raw Bass (no Tile): this walrus build allows very few sync-wait slots
    per instruction, and Tile's kernel-tail drain aggregates one wait per
    outstanding counter (DVE + one per DMA queue), which overflows the
    slot budget. With explicit semaphores every instruction carries at
    most one wait. c rides in the same DMA as x (appended to every row
    on host) so the DVE stream has a single DMA dependency.

    Layout: partition p holds batch rows 2p (g=0) and 2p+1 (g=1), so both
    the input DMA (2560 B/partition) and output DMA (512 B/partition) are
    contiguous per partition. The input DMA is split into 4 partition
    stripes (round-robins onto 4 HW queues), the output into 2.
    The last tree level rides the TensorScalar's second scalar slot:
    out = (c * r_even) * r_odd.
    """
    nc = bass.Bass(use_seq_codegen=True)
    xg = nc.declare_dram_parameter("xg", [B_LOC, D], mybir.dt.float32, isOutput=False)
    cb = nc.declare_dram_parameter("cb", [P, K], mybir.dt.float32, isOutput=False)
    out = nc.declare_dram_parameter("out", [B_LOC, K], mybir.dt.float32, isOutput=True)

    with (
        nc.sbuf_tensor([P, G * D], mybir.dt.float32) as xt,
        nc.sbuf_tensor([P, K], mybir.dt.float32) as ct,
        nc.sbuf_tensor([P, G * (D // 2)], mybir.dt.float32) as ta,
        nc.sbuf_tensor([P, G * (D // 4)], mybir.dt.float32) as tb,
        nc.sbuf_tensor([P, G * K], mybir.dt.float32) as ot,
        nc.semaphore("dsem") as dsem,
        nc.semaphore("csem") as csem,
        nc.semaphore("asem") as asem,
        nc.semaphore("bsem") as bsem,
        nc.semaphore("vsem") as vsem,
        nc.Block() as block,
    ):
        xt_v = xt[:, :].rearrange("p (g c) -> p g c", g=G)
        # Row pairs (2p, 2p+1) fold to one contiguous 2048 B (in) / 512 B
        # (out) line per partition: plain 2D DMAs, no inner strides.
        xg_v = xg[:, :].rearrange("(p two) c -> p (two c)", two=G)
        out_v = out[:, :].rearrange("(p two) k -> p (two k)", two=G)
        H = P // 2     # partition stripe per HWDGE engine
        DTOT = 16 * 2  # 2 out stripes on dsem
        NV_END = 11    # op1a+op1b + 6 tree levels + c-forward + 2 TS

        def io_stream(eng, sl):
            # One HWDGE engine (SP or ACT) moves one partition stripe in
            # two phases (even rows = g0, then odd rows = g1) and, once
            # the DVE signals, back out; both engines run concurrently on
            # their own HW queues. The phase split lets the first tree
            # level start as soon as the g0 rows land.
            eng.dma_start(out=xt[sl, 0:D], in_=xg_v[sl, 0:D]).then_inc(asem, 16)
            eng.dma_start(out=xt[sl, D : 2 * D], in_=xg_v[sl, D : 2 * D]).then_inc(
                bsem, 16
            )
            eng.wait_ge(vsem, NV_END)
            eng.dma_start(out=out_v[sl], in_=ot[sl, :]).then_inc(dsem, 16)
            eng.wait_ge(dsem, DTOT)

        @block.sync
        def _(sync):
            io_stream(sync, slice(0, H))

        @block.scalar
        def _(scalar):
            io_stream(scalar, slice(H, P))

        @block.gpsimd
        def _(gpsimd):
            # c broadcast rides a SWDGE queue (slow: ~2.5us end to end)
            # off the hot HWDGE paths, on its own semaphore so the DVE
            # tree starts on x alone. Its completion is forwarded into
            # the vsem chain after the tree (vsem 8 -> 9), so the first
            # tensor_scalar's single wait slot (vsem >= 9) covers both
            # "tree done" and "c loaded".
            gpsimd.dma_start(out=ct[:, :], in_=cb[:, :]).then_inc(csem, 16)
            gpsimd.wait_ge(csem, 16)
            gpsimd.wait_ge(vsem, 8).then_inc(vsem, 1)

        @block.vector
        def _(vector):
            # Log-tree per-row product: width 256 -> 2 per row. Level 1
            # runs as two half-ops (g0 as soon as phase A lands, g1 on
            # phase B); levels 2..7 process both row groups per op via
            # (p, g, d) views, ping-ponging ta/tb. DVE writes are NOT
            # visible to the next DVE op without a semaphore (measured on
            # HW: dropping these corrupts results), so every dependent op
            # waits on its producer's completion inc; the wait rides the
            # op instruction itself (no standalone waits). op1b writes a
            # region disjoint from op1a, so it needs no vsem wait.
            h = D // 2
            for g in range(G):
                ins = nc.vector.tensor_mul(
                    ta[:, g * h : (g + 1) * h],
                    xt[:, g * D : g * D + h],
                    xt[:, g * D + h : (g + 1) * D],
                )
                ins._wait_ge(asem if g == 0 else bsem, 32)
                ins.then_inc(vsem, 1)
            cur = ta[:, :].rearrange("p (g d) -> p g d", g=G)
            w = h
            k = 2
            scratch = [tb, ta]
            while w > 2:
                h = w // 2
                nxt = scratch[k % 2][:, 0 : G * h].rearrange(
                    "p (g d) -> p g d", g=G
                )
                ins = nc.vector.tensor_mul(nxt, cur[:, :, 0:h], cur[:, :, h:w])
                ins._wait_ge(vsem, k)
                ins.then_inc(vsem, 1)
                k += 1
                cur = nxt
                w = h
            # out[p, g, kk] = (c[kk] * cur[p,g,0]) * cur[p,g,1]
            # (last tree level fused into the tensor_scalar's second op)
            k += 1  # the c-forward's vsem slot sits between tree and TS
            for g in range(G):
                ins = nc.vector.tensor_scalar(
                    out=ot[:, g * K : (g + 1) * K],
                    in0=ct[:, :],
                    scalar1=cur[:, g : g + 1, 0:1],
                    scalar2=cur[:, g : g + 1, 1:2],
                    op0=mybir.AluOpType.mult,
                    op1=mybir.AluOpType.mult,
                )
                ins._wait_ge(vsem, k)
                ins.then_inc(vsem, 1)
                k += 1

    return nc


def _get_bass() -> bass.Bass:
    global _NC_CACHE
    if _NC_CACHE is None:
        _NC_CACHE = _build_bass()
    return _NC_CACHE


def _fold_weights(inputs: dict) -> np.ndarray:
    """Run the weight-only u-recursion (f64) down to the root: c = u_8[0]."""
    u = np.asarray(inputs["w_in"], dtype=np.float64)[:, :, 0]  # (D, K), C == 1
    for l in range(NUM_LEVELS):
        idx = np.asarray(inputs[f"idx{l}"], dtype=np.int64)
        w = np.asarray(inputs[f"w{l}"], dtype=np.float64)
        u = np.einsum("foi,fi->fo", w, u[idx[:, 0]] * u[idx[:, 1]])
    return u[0].astype(np.float32)  # (K,)


def kernel(**inputs: np.ndarray) -> np.ndarray:
    x = np.asarray(inputs["x"], dtype=np.float32)          # (B, 1, D)
    scope = np.asarray(inputs["scope_idx"], dtype=np.int64)[:, 0]

    c = _fold_weights(inputs)                               # (K,) f32
    cb = np.ascontiguousarray(np.broadcast_to(c[None, :], (P, K)))

    # Input-layer bookkeeping gather (leaf scope of the root's product).
    xg = np.ascontiguousarray(x[:, 0, :][:, scope])         # (B, D)

    _ensure_ntff_hook()
    nc = _get_bass()
    in_maps = [
        {"xg": np.ascontiguousarray(xg[i * B_LOC : (i + 1) * B_LOC]), "cb": cb}
        for i in range(N_CORES)
    ]
    res = run_bass_kernel_spmd(
        nc, in_maps, list(range(N_CORES)), trace=TRACE, trace_cores=[0] if TRACE else None
    )
    global LAST_RESULT
    LAST_RESULT = res

    out = np.concatenate([res.results[i]["out"] for i in range(N_CORES)], axis=0)
    return np.ascontiguousarray(out.reshape(B, C, K))


# revision 22
# speedup vs baseline: 1.1443x; 1.0057x over previous
"""Trainium2 Bass kernel for nn_AbstractTorchCircuit_51754355917582.

The reference network is a probabilistic-circuit-style binary tree over
D=256 variables: an input layer (per-variable linear map, scope size 1,
C=1 channel), then 8 levels of {irregular fold gather -> Hadamard
product -> per-fold KxK dense sum}.

Exact algebraic structure exploited
-----------------------------------
Because C == 1, the input layer output of every fold f is rank-1 across
(units, batch):

    h0[f, k, b] = w_in[f, k, 0] * x[b, 0, scope[f]]  =  u0[f, k] * v0[f, b]

and rank-1 structure is preserved *exactly* by both inner-layer ops:

    Hadamard:  (ua*ub)[k] x (va*vb)[b]          (outer product again)
    dense sum: (W @ (ua*ub))[o] x (va*vb)[b]

So with h_l[f] = u_l[f,:] (outer) v_l[f,:], the recursions

    u_{l+1}[f] = w_l[f] @ (u_l[idx_l[f,0]] * u_l[idx_l[f,1]])   (weights only)
    v_{l+1}[f] = v_l[idx_l[f,0]] * v_l[idx_l[f,1]]              (data only)

hold exactly (verified to f64 roundoff against the reference einsums).
Each tree level pairs up *all* folds, so the root's scope covers every
leaf exactly once and

    out[b, 0, k] = c[k] * prod_f x[b, 0, scope[f]],   c = u_8[0]  (K,)

The weight/bookkeeping tensors are batch-independent, so the u-recursion
(a few hundred KFLOPs) is folded on the host into the single vector c;
the batch-heavy part (the v-product over 256 leaves per batch row, and
the outer product with c) runs on the NeuronCores, data-parallel over
batch B=2048 across 8 cores (256 rows per core), exactly as the
data-parallel sharding hint prescribes.

Device kernel (per core)
------------------------
  - DMA the core's (256, 256) slab of gathered x into SBUF as
    (128 partitions, 2 x 256): partition p holds batch rows 2p and 2p+1
    (contiguous 1024 B HBM lines per row block), split across the two
    HWDGE engines (SP / ACT) and phased even-rows-first so the vector
    engine starts early.
  - log-tree DVE multiplies reduce each row to its product r[b]; the
    last level is fused into the two tensor_scalar ops:
    out = (c * r_even_half) * r_odd_half per row group.
  - DMA back to HBM as (256, 64), again striped over both HWDGE engines.

Numerics note: the reference's f32 forward pass underflows to exactly
0.0 everywhere (the activation scale squares at every level:
1e-1 -> 1e-2 -> 1e-4 -> ... -> ~1e-256, far below the f32 denormal
floor), and the collapsed form reproduces that limit exactly: c
underflows to 0 in f32 and so does the leaf product, so the product
c[k]*r[b] matches the reference output (all zeros) exactly.
"""

import sys
import types

import numpy as np

import concourse.bass as bass
import concourse.tile as tile
from concourse import mybir
from concourse.bass_utils import run_bass_kernel_spmd


def _ensure_ntff_hook() -> None:
    """Best-effort: provide ``antenv.axon_hooks`` when the image lacks it.

    ``run_bass_kernel_spmd(trace=True)`` (or BASS_TRACE=1 in the env)
    imports ``antenv.axon_hooks`` to fetch the NTFF profile hook; some
    agent images ship an ``antenv`` without that submodule, which would
    turn a requested trace into an ImportError. Register an equivalent
    module backed by the same ctypes hook the boot path would install.
    No-op if the real module exists or anything is missing.
    """
    try:
        import antenv.axon_hooks  # noqa: F401

        return
    except ImportError:
        pass
    try:
        import antenv
        from trn_agent_boot.trn_boot import _ntff_profile_via_ctypes

        hook = _ntff_profile_via_ctypes("/opt/axon/libaxon_pjrt.so")
        mod = types.ModuleType("antenv.axon_hooks")
        _state = {"hook": hook}
        mod.set_axon_ntff_profile_hook = lambda h: _state.__setitem__("hook", h)
        mod.get_axon_ntff_profile_hook = lambda: _state["hook"]
        sys.modules["antenv.axon_hooks"] = mod
        antenv.axon_hooks = mod
    except Exception:
        pass

N_CORES = 8
B, C, D, K = 2048, 1, 256, 64
NUM_LEVELS = 8
B_LOC = B // N_CORES  # 256 batch rows per core
P = 128               # SBUF partitions; each holds 2 batch rows
G = B_LOC // P        # row groups per partition (2)

# Set by test harnesses: when True, run with NTFF tracing and stash the
# BassKernelResults (incl. exec_time_ns) in LAST_RESULT.
TRACE = False
LAST_RESULT = None

_NC_CACHE = None


def _build_bass() -> bass.Bass:
    """(128, 2x256) x slab -> row products -> scale by c -> (256, 64) out.

    Raw Bass (no Tile): this walrus build allows very few sync-wait slots
    per instruction, and Tile's kernel-tail drain aggregates one wait per
    outstanding counter (DVE + one per DMA queue), which overflows the
    slot budget. With explicit semaphores every instruction carries at
    most one wait.

    Layout: partition p holds batch rows 2p (g=0) and 2p+1 (g=1), so the
    input rows (1024 B each) and the output (512 B/partition) are
    contiguous HBM lines. The input is striped across the two HWDGE
    engines (SP, ACT) and phased g0-rows-first (asem) then g1 (bsem) so
    tree level 1 starts on g0 while g1 is still in flight; the output is
    striped the same way. The c broadcast goes on a (slow) SWDGE queue
    and joins the dependency chain via a GpSimd semaphore forward, so no
    instruction ever needs two waits. The last tree level rides the
    TensorScalar's second scalar slot: out = (c * r_a) * r_b.
    """
    nc = bass.Bass(use_seq_codegen=True)
    xg = nc.declare_dram_parameter("xg", [B_LOC, D], mybir.dt.float32, isOutput=False)
    cb = nc.declare_dram_parameter("cb", [P, K], mybir.dt.float32, isOutput=False)
    out = nc.declare_dram_parameter("out", [B_LOC, K], mybir.dt.float32, isOutput=True)

    with (
        nc.sbuf_tensor([P, G * D], mybir.dt.float32) as xt,
        nc.sbuf_tensor([P, K], mybir.dt.float32) as ct,
        nc.sbuf_tensor([P, G * (D // 2)], mybir.dt.float32) as ta,
        nc.sbuf_tensor([P, G * (D // 4)], mybir.dt.float32) as tb,
        nc.sbuf_tensor([P, G * K], mybir.dt.float32) as ot,
        nc.semaphore("dsem") as dsem,
        nc.semaphore("csem") as csem,
        nc.semaphore("asem") as asem,
        nc.semaphore("bsem") as bsem,
        nc.semaphore("vsem") as vsem,
        nc.Block() as block,
    ):
        xt_v = xt[:, :].rearrange("p (g c) -> p g c", g=G)
        # Row pairs (2p, 2p+1) fold to one contiguous 2048 B (in) / 512 B
        # (out) line per partition: plain 2D DMAs, no inner strides.
        xg_v = xg[:, :].rearrange("(p two) c -> p (two c)", two=G)
        out_v = out[:, :].rearrange("(p two) k -> p (two k)", two=G)
        H = P // 2     # partition stripe per HWDGE engine
        DTOT = 16 * 2  # 2 out stripes on dsem
        NV_END = 11    # op1a+op1b + 6 tree levels + c-forward + 2 TS

        def io_stream(eng, sl):
            # One HWDGE engine (SP or ACT) moves one partition stripe in
            # two phases (even rows = g0, then odd rows = g1) and, once
            # the DVE signals, back out; both engines run concurrently on
            # their own HW queues. The phase split lets the first tree
            # level start as soon as the g0 rows land.
            eng.dma_start(out=xt[sl, 0:D], in_=xg_v[sl, 0:D]).then_inc(asem, 16)
            eng.dma_start(out=xt[sl, D : 2 * D], in_=xg_v[sl, D : 2 * D]).then_inc(
                bsem, 16
            )
            eng.wait_ge(vsem, NV_END)
            eng.dma_start(out=out_v[sl], in_=ot[sl, :]).then_inc(dsem, 16)
            eng.wait_ge(dsem, DTOT)

        @block.sync
        def _(sync):
            io_stream(sync, slice(0, H))

        @block.scalar
        def _(scalar):
            io_stream(scalar, slice(H, P))

        @block.gpsimd
        def _(gpsimd):
            # c broadcast rides a SWDGE queue (slow: ~2.5us end to end)
            # off the hot HWDGE paths, on its own semaphore so the DVE
            # tree starts on x alone. Its completion is forwarded into
            # the vsem chain after the tree (vsem 8 -> 9), so the first
            # tensor_scalar's single wait slot (vsem >= 9) covers both
            # "tree done" and "c loaded".
            gpsimd.dma_start(out=ct[:, :], in_=cb[:, :]).then_inc(csem, 16)
            gpsimd.wait_ge(csem, 16)
            gpsimd.wait_ge(vsem, 8).then_inc(vsem, 1)

        @block.vector
        def _(vector):
            # Log-tree per-row product: width 256 -> 2 per row. Level 1
            # runs as two half-ops (g0 as soon as phase A lands, g1 on
            # phase B); levels 2..7 process both row groups per op via
            # (p, g, d) views, ping-ponging ta/tb. DVE writes are NOT
            # visible to the next DVE op without a semaphore (measured on
            # HW: dropping these corrupts results), so every dependent op
            # waits on its producer's completion inc; the wait rides the
            # op instruction itself (no standalone waits). op1b writes a
            # region disjoint from op1a, so it needs no vsem wait.
            h = D // 2
            for g in range(G):
                ins = nc.vector.tensor_mul(
                    ta[:, g * h : (g + 1) * h],
                    xt[:, g * D : g * D + h],
                    xt[:, g * D + h : (g + 1) * D],
                )
                ins._wait_ge(asem if g == 0 else bsem, 32)
                ins.then_inc(vsem, 1)
            cur = ta[:, :].rearrange("p (g d) -> p g d", g=G)
            w = h
            k = 2
            scratch = [tb, ta]
            while w > 2:
                h = w // 2
                nxt = scratch[k % 2][:, 0 : G * h].rearrange(
                    "p (g d) -> p g d", g=G
                )
                ins = nc.vector.tensor_mul(nxt, cur[:, :, 0:h], cur[:, :, h:w])
                ins._wait_ge(vsem, k)
                ins.then_inc(vsem, 1)
                k += 1
                cur = nxt
                w = h
            # out[p, g, kk] = (c[kk] * cur[p,g,0]) * cur[p,g,1]
            # (last tree level fused into the tensor_scalar's second op)
            k += 1  # the c-forward's vsem slot sits between tree and TS
            for g in range(G):
                ins = nc.vector.tensor_scalar(
                    out=ot[:, g * K : (g + 1) * K],
                    in0=ct[:, :],
                    scalar1=cur[:, g : g + 1, 0:1],
                    scalar2=cur[:, g : g + 1, 1:2],
                    op0=mybir.AluOpType.mult,
                    op1=mybir.AluOpType.mult,
                )
                ins._wait_ge(vsem, k)
                ins.then_inc(vsem, 1)
                k += 1

    return nc


def _get_bass() -> bass.Bass:
    global _NC_CACHE
    if _NC_CACHE is None:
        _NC_CACHE = _build_bass()
    return _NC_CACHE


def _fold_weights(inputs: dict) -> np.ndarray:
    """Run the weight-only u-recursion (f64) down to the root: c = u_8[0]."""
    u = np.asarray(inputs["w_in"], dtype=np.float64)[:, :, 0]  # (D, K), C == 1
    for l in range(NUM_LEVELS):
        idx = np.asarray(inputs[f"idx{l}"], dtype=np.int64)
        w = np.asarray(inputs[f"w{l}"], dtype=np.float64)
        u = np.einsum("foi,fi->fo", w, u[idx[:, 0]] * u[idx[:, 1]])
    return u[0].astype(np.float32)  # (K,)


def kernel(**inputs: np.ndarray) -> np.ndarray:
    x = np.asarray(inputs["x"], dtype=np.float32)          # (B, 1, D)
    scope = np.asarray(inputs["scope_idx"], dtype=np.int64)[:, 0]

    c = _fold_weights(inputs)                               # (K,) f32
    cb = np.ascontiguousarray(np.broadcast_to(c[None, :], (P, K)))

    # Input-layer bookkeeping gather (leaf scope of the root's product).
    xg = np.ascontiguousarray(x[:, 0, :][:, scope])         # (B, D)

    _ensure_ntff_hook()
    nc = _get_bass()
    in_maps = [
        {"xg": np.ascontiguousarray(xg[i * B_LOC : (i + 1) * B_LOC]), "cb": cb}
        for i in range(N_CORES)
    ]
    res = run_bass_kernel_spmd(
        nc, in_maps, list(range(N_CORES)), trace=TRACE, trace_cores=[0] if TRACE else None
    )
    global LAST_RESULT
    LAST_RESULT = res

    out = np.concatenate([res.results[i]["out"] for i in range(N_CORES)], axis=0)
    return np.ascontiguousarray(out.reshape(B, C, K))
